# revision 1
# baseline (speedup 1.0000x reference)
"""Trainium2 Bass kernel for nn_Chord_Note_Conv (ragged embedding-bag + conv1d stack).

Design (per core, data-parallel over batch: B=16 -> 2 batch rows/core, P=4096 positions):
  * The ragged note-embedding mean, the chord embedding lookup, AND conv1 are fused
    into sparse-histogram matmuls: for each position build a 1024-bin count row
    C'[pos, v] (note bins 0..831 scaled by 1/cnt, chord bin 832+chord = 1),
    then conv1(x) = sum_k C'[s+k-1, :] @ F_k with F_k = E'' @ W1_k^T precomputed
    on host ([1024, 256] per tap).
  * C' rows are built with the GPSIMD local_scatter instruction (per-partition
    independent scatter; duplicates resolved on DVE via a shift-compare pass),
    transposed to matmul lhsT/rhs layout with the HWDGE xbar DMA transpose.
  * conv2 + fc are plain PE matmuls; fc bias via an appended ones-row.
  * Everything on-chip is fp16 (fp32 PSUM accumulation) -> ~1e-4 rel err.
"""

import os

os.environ.setdefault("MYCRO_LOCAL_CACHE", "1")

import numpy as np

try:
    import concourse.bass as bass  # noqa: F401
except ImportError:
    import sys

    sys.path.insert(0, "/opt/trn_rl_repo")
    import concourse.bass as bass

from concourse import bacc, mybir, tile
from concourse.bass_utils import run_bass_kernel_spmd

FP16 = mybir.dt.float16   # NB: bf16 produced wrong results on HW (and was no faster)
F32 = mybir.dt.float32
I16 = mybir.dt.int16
ALU = mybir.AluOpType

CHORD_SIZE, NOTE_SIZE = 150, 832
B, S, N = 16, 2048, 16
NCORES = 8
BLOC = B // NCORES          # batch rows per core
P = BLOC * S                # positions per core = 4096
NT = P // 128               # pos tiles per core = 32
NSB = P // 512              # s-blocks per core = 8
SB_PER_BATCH = S // 512     # 4
NBINS = 1024                # 832 note + 150 chord + pad
NQ = NBINS // 128           # 8 v-chunks
CW = 544                    # ct/x2 s-block buffer width (16 guard + 512 + pad)


def _build_program():
    nc = bacc.Bacc("TRN2", target_bir_lowering=False, debug=False,
                   enable_asserts=False, num_devices=NCORES)

    # ---- DRAM I/O (flat 2D) ----
    d_note = nc.dram_tensor("note16", [128, NT * 16], I16, kind="ExternalInput")
    d_chord = nc.dram_tensor("chord16", [128, NT], I16, kind="ExternalInput")
    d_f1t = nc.dram_tensor("f1t", [128, 3 * NQ * 2 * 128], FP16, kind="ExternalInput")
    d_w2t = nc.dram_tensor("w2t", [128, 3 * 2 * 64], FP16, kind="ExternalInput")
    d_fcwb = nc.dram_tensor("fcwb", [65, 152], FP16, kind="ExternalInput")
    d_b1 = nc.dram_tensor("b1t", [128, 2], F32, kind="ExternalInput")
    d_b2 = nc.dram_tensor("b2t", [64, 1], F32, kind="ExternalInput")
    d_prepc = nc.dram_tensor("prepc", [128, 528], I16, kind="ExternalInput")
    d_out = nc.dram_tensor("out", [P, CHORD_SIZE], F32, kind="ExternalOutput")

    # ---- persistent SBUF ----
    def sb(name, shape, dt):
        return nc.alloc_sbuf_tensor(name, list(shape), dt).ap()

    s_note = sb("s_note", [128, NT * 16], I16)
    s_chord = sb("s_chord", [128, NT], I16)
    s_f1t = sb("s_f1t", [128, 3 * NQ * 2 * 128], FP16)
    s_w2t = sb("s_w2t", [128, 3 * 2 * 64], FP16)
    s_fcwb = sb("s_fcwb", [65, 152], FP16)
    s_b1 = sb("s_b1", [128, 2], F32)
    s_b2 = sb("s_b2", [64, 1], F32)
    s_prepc = sb("s_prepc", [128, 528], I16)
    s_eqall = sb("s_eqall", [128, 8 * 256], I16)
    s_eqall2 = sb("s_eqall2", [128, 8 * 256], I16)
    s_x3 = sb("s_x3", [65, P], FP16)
    s_mask = sb("s_mask", [128, NT * 16], I16)
    s_val = sb("s_val", [128, NT * 16], I16)
    s_isdup = sb("s_isdup", [128, NT * 16], I16)
    s_eq = sb("s_eq", [128, NT * 16], I16)
    s_tmp = sb("s_tmp", [128, NT * 16], I16)
    s_v16 = sb("s_v16", [128, NT * 16], FP16)
    s_cnt = sb("s_cnt", [128, NT], F32)
    s_inv = sb("s_inv", [128, NT], F32)
    s_inv16 = sb("s_inv16", [128, NT], FP16)
    s_sidx = sb("s_sidx", [128, NT * 18], I16)
    s_sval = sb("s_sval", [128, NT * 18], FP16)
    # 8 ct s-block buffers [128, NQ, CW] + 8 x2 buffers [128, 2, CW]
    s_ct = [sb(f"s_ct{i}", [128, NQ * CW], FP16) for i in range(NSB)]
    s_x2 = [sb(f"s_x2{i}", [128, 2 * CW], FP16) for i in range(NSB)]

    note3 = s_note.rearrange("p (c j) -> p c j", j=16)
    mask3 = s_mask.rearrange("p (c j) -> p c j", j=16)
    val3 = s_val.rearrange("p (c j) -> p c j", j=16)
    isd3 = s_isdup.rearrange("p (c j) -> p c j", j=16)
    eq3 = s_eq.rearrange("p (c j) -> p c j", j=16)
    tmp3 = s_tmp.rearrange("p (c j) -> p c j", j=16)
    v163 = s_v16.rearrange("p (c j) -> p c j", j=16)
    sidx3 = s_sidx.rearrange("p (c j) -> p c j", j=18)
    sval3 = s_sval.rearrange("p (c j) -> p c j", j=18)
    f1t5 = s_f1t.rearrange("p (k q c o) -> p k q c o", k=3, q=NQ, c=2)
    w2t4 = s_w2t.rearrange("p (k q o) -> p k q o", k=3, q=2)
    ct3 = [t.rearrange("p (q w) -> p q w", w=CW) for t in s_ct]
    x23 = [t.rearrange("p (q w) -> p q w", w=CW) for t in s_x2]

    with tile.TileContext(nc) as tc, \
         nc.allow_low_precision(reason="int16 counts <=16 are exact; fp16 data"):
        v = nc.vector
        # ---- input loads ----
        nc.sync.dma_start(s_note, d_note.ap())
        nc.sync.dma_start(s_chord, d_chord.ap())
        nc.sync.dma_start(s_f1t, d_f1t.ap())
        nc.sync.dma_start(s_w2t, d_w2t.ap())
        nc.sync.dma_start(s_fcwb, d_fcwb.ap())
        nc.sync.dma_start(s_b1, d_b1.ap())
        nc.sync.dma_start(s_b2, d_b2.ap())
        nc.sync.dma_start(s_prepc, d_prepc.ap())

        # ---- constants ----
        nc.gpsimd.memset(s_x3[64:65, :], 1.0)  # fc ones row (Pool is idle early)
        v.memset(sidx3[:, :, 17:18], -1)       # pad slot ignored
        v.memset(sval3[:, :, 16:17], 1.0)      # chord weight
        v.memset(sval3[:, :, 17:18], 0.0)

        # ---- index prep (DVE), c-range granular so scatters start early ----
        def prep(a, b, after=None):
            w_ = b - a
            nt = note3[:, a:b, :]
            mk = mask3[:, a:b, :]
            vl = val3[:, a:b, :]
            isd = isd3[:, a:b, :]
            eq = eq3[:, a:b, :]
            tp = tmp3[:, a:b, :]
            first = v.tensor_scalar(mk, nt, 0, None, ALU.not_equal)
            if after is not None:
                tile.add_dep_helper(first.ins, after.ins, sync=False,
                                    reason="bulk prep after head prep")
            for j in range(1, 16):
                v.tensor_tensor(mk[:, :, j], mk[:, :, j - 1], mk[:, :, j], ALU.mult)
            v.reduce_sum(s_cnt[:, a:b], mk, axis=mybir.AxisListType.X)
            v.tensor_scalar_max(s_cnt[:, a:b], s_cnt[:, a:b], 1.0)
            v.reciprocal(s_inv[:, a:b], s_cnt[:, a:b])
            v.tensor_copy(s_inv16[:, a:b], s_inv[:, a:b])
            v.tensor_copy(vl, mk)
            v.memset(isd, 0)
            for d in range(1, 16):
                w = 16 - d
                v.tensor_tensor(eq[:, :, :w], nt[:, :, d:], nt[:, :, :w], ALU.is_equal)
                v.tensor_tensor(tp[:, :, :w], mk[:, :, d:], eq[:, :, :w], ALU.mult)
                v.tensor_tensor(vl[:, :, :w], vl[:, :, :w], tp[:, :, :w], ALU.add)
                v.tensor_tensor(isd[:, :, d:], isd[:, :, d:], eq[:, :, :w], ALU.max)
            return finish_prep(a, b)

        def finish_prep(a, b):
            """common tail: fv, scat_idx, scat_val from mask/val/isdup."""
            w_ = b - a
            nt = note3[:, a:b, :]
            mk = mask3[:, a:b, :]
            isd = isd3[:, a:b, :]
            eq = eq3[:, a:b, :]
            tp = tmp3[:, a:b, :]
            v.tensor_tensor(tp, mk, isd, ALU.mult)
            v.tensor_tensor(isd, mk, tp, ALU.subtract)   # fv
            v.tensor_tensor(tp, nt, isd, ALU.mult)
            v.tensor_scalar_add(eq, isd, -1)
            v.tensor_tensor(sidx3[:, a:b, 0:16], tp, eq, ALU.add)
            v.tensor_copy(sidx3[:, a:b, 16:17], s_chord[:, a:b].unsqueeze(2))
            v.tensor_copy(v163[:, a:b, :], val3[:, a:b, :])
            inv_b = s_inv16[:, a:b].unsqueeze(2).broadcast_to((128, w_, 16))
            return v.tensor_tensor(sval3[:, a:b, 0:16], v163[:, a:b, :], inv_b, ALU.mult)

        ut2 = s_prepc[:, 0:256].rearrange("p (a b) -> p a b", b=16)    # 1[j' >= j]
        lt2 = s_prepc[:, 256:512].rearrange("p (a b) -> p a b", b=16)  # 1[j' < j]
        jidx = s_prepc[:, 512:528]                                     # iota 0..15

        def head_prep(a, b):
            """Low-latency variant: 16x16 outer compare in few big ops."""
            w_ = b - a
            nt = note3[:, a:b, :]
            mk = mask3[:, a:b, :]
            eq = eq3[:, a:b, :]
            tp = tmp3[:, a:b, :]
            eqa = s_eqall.rearrange("p (c i j) -> p c i j", i=16, j=16)[:, 0:w_, :, :]
            eqb = s_eqall2.rearrange("p (c i j) -> p c i j", i=16, j=16)[:, 0:w_, :, :]
            jid_b = jidx.unsqueeze(1).broadcast_to((128, w_, 16))
            # first-zero index -> mask, cnt
            v.tensor_scalar(eq, nt, 0, None, ALU.not_equal)            # nz
            v.scalar_tensor_tensor(tp, eq, 16, jid_b, ALU.mult, ALU.add)
            v.tensor_reduce(s_cnt[:, a:b], tp, mybir.AxisListType.X, ALU.min)
            v.tensor_tensor(mk, jid_b,
                            s_cnt[:, a:b].unsqueeze(2).broadcast_to((128, w_, 16)),
                            ALU.is_lt)
            v.tensor_scalar(s_cnt[:, a:b], s_cnt[:, a:b], 16.0, 1.0, ALU.min, ALU.max)
            v.reciprocal(s_inv[:, a:b], s_cnt[:, a:b])
            v.tensor_copy(s_inv16[:, a:b], s_inv[:, a:b])
            # 16x16 equality outer product
            v.tensor_tensor(eqa,
                            nt.unsqueeze(3).broadcast_to((128, w_, 16, 16)),
                            nt.unsqueeze(2).broadcast_to((128, w_, 16, 16)),
                            ALU.is_equal)
            # t = m_j' * eq ; val_j = sum_{j'>=j} t ; isdup_j = max_{j'<j} t
            # (valid-slot dup of an earlier masked slot is impossible: mask is
            #  a prefix, so masking t for isdup too is equivalent and reuses it)
            v.tensor_tensor(eqa, eqa,
                            mask3[:, a:b, :].unsqueeze(2).broadcast_to((128, w_, 16, 16)),
                            ALU.mult)
            v.tensor_tensor(eqb, eqa,
                            lt2.unsqueeze(1).broadcast_to((128, w_, 16, 16)), ALU.mult)
            v.tensor_reduce(isd3[:, a:b, :], eqb, mybir.AxisListType.X, ALU.max)
            v.tensor_tensor(eqa, eqa,
                            ut2.unsqueeze(1).broadcast_to((128, w_, 16, 16)), ALU.mult)
            v.tensor_reduce(val3[:, a:b, :], eqa, mybir.AxisListType.X, ALU.add)
            return finish_prep(a, b)

        # ---- histogram + transpose ----
        def hist_sb(cpool, sbk):
            eng = nc.sync
            for t in range(4):
                ti = 4 * sbk + t
                ct_ = cpool.tile([128, NBINS], FP16, tag="c")
                nc.gpsimd.local_scatter(
                    ct_[:], sval3[:, ti, :], sidx3[:, ti, :],
                    channels=128, num_elems=NBINS, num_idxs=18)
                eng.dma_start_transpose(
                    ct3[sbk][:, :, 16 + 128 * t: 144 + 128 * t], ct_[:])
            if sbk % SB_PER_BATCH == 0:
                v.memset(ct3[sbk][:, :, 15:16], 0.0)
            else:
                v.tensor_copy(ct3[sbk][:, :, 15:16], ct3[sbk - 1][:, :, 527:528])

        with tc.tile_pool(name="cp", bufs=4) as cpool:
            h1 = head_prep(0, 8)
            hist_sb(cpool, 0)
            hist_sb(cpool, 1)
            prep(8, NT, after=h1)
            for sbk in range(2, NSB):
                hist_sb(cpool, sbk)

            # ---- conv1 / conv2 / fc : software-pipelined over s-blocks ----
            with tc.tile_pool(name="p1", bufs=3, space="PSUM") as pp, \
                 tc.tile_pool(name="p2", bufs=2, space="PSUM") as pp2, \
                 tc.tile_pool(name="pf", bufs=3, space="PSUM") as pf, \
                 tc.tile_pool(name="ob", bufs=6) as ob:

                def conv1_sb(sbk):
                    if sbk % SB_PER_BATCH == SB_PER_BATCH - 1:
                        v.memset(ct3[sbk][:, :, 528:529], 0.0)
                    else:
                        v.tensor_copy(ct3[sbk][:, :, 528:529], ct3[sbk + 1][:, :, 16:17])
                    for co in range(2):
                        ps = pp.tile([128, 512], F32, tag="ps")
                        mms = [(1, 0)] + [(k, q) for k in range(3) for q in range(NQ)
                                          if not (k == 1 and q == 0)]
                        for i, (k, q) in enumerate(mms):
                            nc.tensor.matmul(
                                ps[:], f1t5[:, k, q, co, :],
                                ct3[sbk][:, q, 15 + k: 527 + k],
                                start=(i == 0), stop=(i == len(mms) - 1))
                        nc.scalar.activation(
                            x23[sbk][:, co, 16:528], ps[:],
                            mybir.ActivationFunctionType.Relu,
                            bias=s_b1[:, co:co + 1])
                    if sbk % SB_PER_BATCH == 0:
                        v.memset(x23[sbk][:, :, 15:16], 0.0)
                    else:
                        v.tensor_copy(x23[sbk][:, :, 15:16], x23[sbk - 1][:, :, 527:528])

                def conv2_sb(sbk):
                    if sbk % SB_PER_BATCH == SB_PER_BATCH - 1:
                        v.memset(x23[sbk][:, :, 528:529], 0.0)
                    else:
                        v.tensor_copy(x23[sbk][:, :, 528:529], x23[sbk + 1][:, :, 16:17])
                    ps2 = pp2.tile([64, 512], F32, tag="ps2")
                    mms = [(1, 0), (0, 0), (2, 0), (0, 1), (1, 1), (2, 1)]
                    for i, (k, q) in enumerate(mms):
                        nc.tensor.matmul(
                            ps2[:], w2t4[:, k, q, :],
                            x23[sbk][:, q, 15 + k: 527 + k],
                            start=(i == 0), stop=(i == len(mms) - 1))
                    nc.scalar.activation(
                        s_x3[0:64, 512 * sbk: 512 * (sbk + 1)], ps2[:],
                        mybir.ActivationFunctionType.Relu, bias=s_b2[:, 0:1])

                def fc_sb(sbk):
                    for t in range(4 * sbk, 4 * sbk + 4):
                        psf = pf.tile([128, CHORD_SIZE], F32, tag="psf")
                        nc.tensor.matmul(psf[:], s_x3[:, 128 * t: 128 * (t + 1)],
                                         s_fcwb[:, 0:CHORD_SIZE], start=True, stop=True)
                        o = ob.tile([128, CHORD_SIZE], F32, tag="o")
                        nc.scalar.copy(o[:], psf[:])
                        nc.sync.dma_start(d_out.ap()[128 * t: 128 * (t + 1), :], o[:])

                for sbk in range(NSB):
                    conv1_sb(sbk)
                    if sbk >= 2:
                        conv2_sb(sbk - 2)
                    if sbk >= 3:
                        fc_sb(sbk - 3)
                for sbk in range(NSB - 2, NSB):
                    conv2_sb(sbk)
                for sbk in range(NSB - 3, NSB):
                    fc_sb(sbk)

    nc.compile()
    return nc


_NC = None


def _get_nc():
    global _NC
    if _NC is None:
        _NC = _build_program()
    return _NC


def _host_prep(chord_emb, note_emb, conv1_w, conv1_b, conv2_w, conv2_b, fc_w, fc_b):
    """Shared (replicated) constant tensors."""
    note_emb = np.asarray(note_emb, np.float32)
    chord_emb = np.asarray(chord_emb, np.float32)
    w1 = np.asarray(conv1_w, np.float32)
    E = np.zeros((NBINS, 512), np.float32)
    E[0:NOTE_SIZE, 256:512] = note_emb
    E[NOTE_SIZE:NOTE_SIZE + CHORD_SIZE, 0:256] = chord_emb
    F = np.einsum('vc,ock->kvo', E, w1)                      # [3, 1024, 256]
    f1t = F.reshape(3, NQ, 128, 2, 128).transpose(2, 0, 1, 3, 4)  # [128,3,q,c,o]
    f1t = np.ascontiguousarray(f1t, np.float16).reshape(128, -1)

    w2 = np.asarray(conv2_w, np.float32).reshape(64, 2, 128, 3)
    w2t = np.ascontiguousarray(w2.transpose(2, 3, 1, 0), np.float16).reshape(128, -1)

    fcwb = np.zeros((65, 152), np.float16)
    fcwb[0:64, 0:CHORD_SIZE] = np.asarray(fc_w, np.float16)
    fcwb[64, 0:CHORD_SIZE] = np.asarray(fc_b, np.float16)

    b1t = np.ascontiguousarray(
        np.asarray(conv1_b, np.float32).reshape(2, 128).T)
    b2t = np.asarray(conv2_b, np.float32).reshape(64, 1)

    jj = np.arange(16, dtype=np.int16)
    ut = (jj[None, :] >= jj[:, None]).astype(np.int16).reshape(-1)   # j' >= j
    lt = (jj[None, :] < jj[:, None]).astype(np.int16).reshape(-1)    # j' < j
    prepc = np.zeros((128, 528), np.int16)
    prepc[:, 0:256] = ut[None, :]
    prepc[:, 256:512] = lt[None, :]
    prepc[:, 512:528] = jj[None, :]
    return f1t, w2t, fcwb, b1t, b2t, prepc


def kernel(chord_emb, note_emb, conv1_w, conv1_b, conv2_w, conv2_b, fc_w, fc_b,
           note, chord):
    nc = _get_nc()
    f1t, w2t, fcwb, b1t, b2t, prepc = _host_prep(
        chord_emb, note_emb, conv1_w, conv1_b, conv2_w, conv2_b, fc_w, fc_b)

    note = np.asarray(note)
    chord = np.asarray(chord)
    in_maps = []
    for c in range(NCORES):
        nf = note[BLOC * c: BLOC * (c + 1)].reshape(P, N).astype(np.int16)
        cf = chord[BLOC * c: BLOC * (c + 1)].reshape(P).astype(np.int16) + NOTE_SIZE
        note16 = np.ascontiguousarray(
            nf.reshape(NT, 128, 16).transpose(1, 0, 2)).reshape(128, -1)
        chord16 = np.ascontiguousarray(cf.reshape(NT, 128).T)
        in_maps.append({
            "note16": note16, "chord16": chord16, "f1t": f1t, "w2t": w2t,
            "fcwb": fcwb, "b1t": b1t, "b2t": b2t, "prepc": prepc,
        })

    res = run_bass_kernel_spmd(nc, in_maps, list(range(NCORES)))
    outs = [res.results[c]["out"].reshape(BLOC, S, CHORD_SIZE)
            for c in range(NCORES)]
    return np.concatenate(outs, axis=0).astype(np.float32)



# revision 7
# speedup vs baseline: 1.0733x; 1.0733x over previous
"""Trainium2 Bass kernel for nn_Chord_Note_Conv (ragged embedding-bag + conv1d stack).

Design (per core, data-parallel over batch: B=16 -> 2 batch rows/core, P=4096 positions):
  * The ragged note-embedding mean, the chord embedding lookup, AND conv1 are fused
    into sparse-histogram matmuls: for each position build a 1024-bin count row
    C'[pos, v] (note bins 0..831 scaled by 1/cnt, chord bin 832+chord = 1),
    then conv1(x) = sum_k C'[s+k-1, :] @ F_k with F_k = E'' @ W1_k^T precomputed
    on host ([1024, 256] per tap).
  * C' rows are built with the GPSIMD local_scatter instruction (per-partition
    independent scatter; duplicates resolved on DVE via a shift-compare pass),
    transposed to matmul lhsT/rhs layout with the HWDGE xbar DMA transpose.
  * conv2 + fc are plain PE matmuls; fc bias via an appended ones-row.
  * Everything on-chip is fp16 (fp32 PSUM accumulation) -> ~1e-4 rel err.
"""

import os

os.environ.setdefault("MYCRO_LOCAL_CACHE", "1")

import numpy as np

try:
    import concourse.bass as bass  # noqa: F401
except ImportError:
    import sys

    sys.path.insert(0, "/opt/trn_rl_repo")
    import concourse.bass as bass

from concourse import bacc, mybir, tile
from concourse.bass_utils import run_bass_kernel_spmd

FP16 = mybir.dt.float16   # NB: bf16 produced wrong results on HW (and was no faster)
F32 = mybir.dt.float32
I16 = mybir.dt.int16
ALU = mybir.AluOpType

CHORD_SIZE, NOTE_SIZE = 150, 832
B, S, N = 16, 2048, 16
NCORES = 8
BLOC = B // NCORES          # batch rows per core
P = BLOC * S                # positions per core = 4096
NT = P // 128               # pos tiles per core = 32
NSB = P // 512              # s-blocks per core = 8
SB_PER_BATCH = S // 512     # 4
NBINS = 1024                # 832 note + 150 chord + pad
NQ = NBINS // 128           # 8 v-chunks
CW = 544                    # ct/x2 s-block buffer width (16 guard + 512 + pad)


def _build_program():
    nc = bacc.Bacc("TRN2", target_bir_lowering=False, debug=False,
                   enable_asserts=False, num_devices=NCORES)

    # ---- DRAM I/O (flat 2D) ----
    d_note = nc.dram_tensor("note16", [128, NT * 16], I16, kind="ExternalInput")
    d_chord = nc.dram_tensor("chord16", [128, NT], I16, kind="ExternalInput")
    d_f1t = nc.dram_tensor("f1t", [128, 3 * NQ * 2 * 128], FP16, kind="ExternalInput")
    d_w2t = nc.dram_tensor("w2t", [128, 3 * 2 * 64], FP16, kind="ExternalInput")
    d_fcwb = nc.dram_tensor("fcwb", [65, 152], FP16, kind="ExternalInput")
    d_b1 = nc.dram_tensor("b1t", [128, 2], F32, kind="ExternalInput")
    d_b2 = nc.dram_tensor("b2t", [64, 1], F32, kind="ExternalInput")
    d_prepc = nc.dram_tensor("prepc", [128, 528], I16, kind="ExternalInput")
    d_out = nc.dram_tensor("out", [P, CHORD_SIZE], F32, kind="ExternalOutput")

    # ---- persistent SBUF ----
    def sb(name, shape, dt):
        return nc.alloc_sbuf_tensor(name, list(shape), dt).ap()

    s_note = sb("s_note", [128, NT * 16], I16)
    s_chord = sb("s_chord", [128, NT], I16)
    s_f1t = sb("s_f1t", [128, 3 * NQ * 2 * 128], FP16)
    s_w2t = sb("s_w2t", [128, 3 * 2 * 64], FP16)
    s_fcwb = sb("s_fcwb", [65, 152], FP16)
    s_b1 = sb("s_b1", [128, 2], F32)
    s_b2 = sb("s_b2", [64, 1], F32)
    s_prepc = sb("s_prepc", [128, 528], I16)
    s_eqall = sb("s_eqall", [128, 8 * 256], I16)
    s_x3 = sb("s_x3", [65, P], FP16)
    s_mask = sb("s_mask", [128, NT * 16], I16)
    s_val = sb("s_val", [128, NT * 16], I16)
    s_eq = sb("s_eq", [128, NT * 16], I16)
    s_tmp = sb("s_tmp", [128, NT * 16], I16)
    s_v16 = sb("s_v16", [128, NT * 16], FP16)
    s_cnt = sb("s_cnt", [128, NT], F32)
    s_inv = sb("s_inv", [128, NT], F32)
    s_inv16 = sb("s_inv16", [128, NT], FP16)
    s_sidx = sb("s_sidx", [128, NT * 18], I16)
    s_sval = sb("s_sval", [128, NT * 18], FP16)
    # 8 ct s-block buffers [128, NQ, CW] + 8 x2 buffers [128, 2, CW]
    s_ct = [sb(f"s_ct{i}", [128, NQ * CW], FP16) for i in range(NSB)]
    s_x2 = [sb(f"s_x2{i}", [128, 2 * CW], FP16) for i in range(NSB)]

    note3 = s_note.rearrange("p (c j) -> p c j", j=16)
    mask3 = s_mask.rearrange("p (c j) -> p c j", j=16)
    val3 = s_val.rearrange("p (c j) -> p c j", j=16)
    eq3 = s_eq.rearrange("p (c j) -> p c j", j=16)
    tmp3 = s_tmp.rearrange("p (c j) -> p c j", j=16)
    v163 = s_v16.rearrange("p (c j) -> p c j", j=16)
    sidx3 = s_sidx.rearrange("p (c j) -> p c j", j=18)
    sval3 = s_sval.rearrange("p (c j) -> p c j", j=18)
    f1t5 = s_f1t.rearrange("p (k q c o) -> p k q c o", k=3, q=NQ, c=2)
    w2t4 = s_w2t.rearrange("p (k q o) -> p k q o", k=3, q=2)
    ct3 = [t.rearrange("p (q w) -> p q w", w=CW) for t in s_ct]
    x23 = [t.rearrange("p (q w) -> p q w", w=CW) for t in s_x2]

    with tile.TileContext(nc) as tc, \
         nc.allow_low_precision(reason="int16 counts <=16 are exact; fp16 data"):
        v = nc.vector
        # ---- input loads: prep-critical on sync queue, bulk weights on the
        # scalar (Activation) HWDGE queue so they load in parallel ----
        nc.sync.dma_start(s_note, d_note.ap())
        nc.sync.dma_start(s_chord, d_chord.ap())
        nc.sync.dma_start(s_prepc, d_prepc.ap())
        nc.scalar.dma_start(s_f1t, d_f1t.ap())
        nc.scalar.dma_start(s_w2t, d_w2t.ap())
        nc.scalar.dma_start(s_fcwb, d_fcwb.ap())
        nc.scalar.dma_start(s_b1, d_b1.ap())
        nc.scalar.dma_start(s_b2, d_b2.ap())

        # ---- constants ----
        nc.gpsimd.memset(s_x3[64:65, :], 1.0)  # fc ones row (Pool is idle early)
        v.memset(sidx3[:, :, 17:18], -1)       # pad slot ignored
        v.memset(sval3[:, :, 16:17], 1.0)      # chord weight
        v.memset(sval3[:, :, 17:18], 0.0)

        # Duplicate notes: slot j scatters the cumulative count of note_j over
        # slots j' <= j; the scatter applies indices in order, so the last
        # occurrence wins and leaves the total count in the bin.
        def prep(a, b, after=None):
            w_ = b - a
            nt = note3[:, a:b, :]
            mk = mask3[:, a:b, :]
            vl = val3[:, a:b, :]
            eq = eq3[:, a:b, :]
            first = v.tensor_scalar(mk, nt, 0, None, ALU.not_equal)
            if after is not None:
                tile.add_dep_helper(first.ins, after.ins, sync=False,
                                    reason="bulk prep after head prep")
            for j in range(1, 16):
                v.tensor_tensor(mk[:, :, j], mk[:, :, j - 1], mk[:, :, j], ALU.mult)
            v.reduce_sum(s_cnt[:, a:b], mk, axis=mybir.AxisListType.X)
            v.tensor_scalar_max(s_cnt[:, a:b], s_cnt[:, a:b], 1.0)
            v.reciprocal(s_inv[:, a:b], s_cnt[:, a:b])
            v.tensor_copy(s_inv16[:, a:b], s_inv[:, a:b])
            v.tensor_copy(vl, mk)
            for d in range(1, 16):
                v.tensor_tensor(eq[:, :, d:], nt[:, :, d:], nt[:, :, :16 - d],
                                ALU.is_equal)
                v.tensor_tensor(vl[:, :, d:], vl[:, :, d:], eq[:, :, d:], ALU.add)
            return finish_prep(a, b)

        def finish_prep(a, b):
            """common tail: scat_idx, scat_val from mask/val."""
            w_ = b - a
            nt = note3[:, a:b, :]
            mk = mask3[:, a:b, :]
            eq = eq3[:, a:b, :]
            tp = tmp3[:, a:b, :]
            v.tensor_tensor(tp, nt, mk, ALU.mult)
            v.tensor_scalar_add(eq, mk, -1)
            v.tensor_tensor(sidx3[:, a:b, 0:16], tp, eq, ALU.add)
            v.tensor_copy(sidx3[:, a:b, 16:17], s_chord[:, a:b].unsqueeze(2))
            v.tensor_copy(v163[:, a:b, :], val3[:, a:b, :])
            inv_b = s_inv16[:, a:b].unsqueeze(2).broadcast_to((128, w_, 16))
            return v.tensor_tensor(sval3[:, a:b, 0:16], v163[:, a:b, :], inv_b, ALU.mult)

        le2 = s_prepc[:, 0:256].rearrange("p (a b) -> p a b", b=16)    # 1[j' <= j]
        jidx = s_prepc[:, 512:528]                                     # iota 0..15

        def head_prep(a, b):
            """Low-latency variant: 16x16 outer compare in few big ops."""
            w_ = b - a
            nt = note3[:, a:b, :]
            mk = mask3[:, a:b, :]
            eq = eq3[:, a:b, :]
            tp = tmp3[:, a:b, :]
            eqa = s_eqall.rearrange("p (c i j) -> p c i j", i=16, j=16)[:, 0:w_, :, :]
            jid_b = jidx.unsqueeze(1).broadcast_to((128, w_, 16))
            # first-zero index -> mask, cnt
            v.tensor_scalar(eq, nt, 0, None, ALU.not_equal)            # nz
            v.scalar_tensor_tensor(tp, eq, 16, jid_b, ALU.mult, ALU.add)
            v.tensor_reduce(s_cnt[:, a:b], tp, mybir.AxisListType.X, ALU.min)
            v.tensor_tensor(mk, jid_b,
                            s_cnt[:, a:b].unsqueeze(2).broadcast_to((128, w_, 16)),
                            ALU.is_lt)
            v.tensor_scalar(s_cnt[:, a:b], s_cnt[:, a:b], 16.0, 1.0, ALU.min, ALU.max)
            v.reciprocal(s_inv[:, a:b], s_cnt[:, a:b])
            v.tensor_copy(s_inv16[:, a:b], s_inv[:, a:b])
            # 16x16 equality outer product -> cumulative dup count over j' <= j
            v.tensor_tensor(eqa,
                            nt.unsqueeze(3).broadcast_to((128, w_, 16, 16)),
                            nt.unsqueeze(2).broadcast_to((128, w_, 16, 16)),
                            ALU.is_equal)
            v.tensor_tensor(eqa, eqa,
                            le2.unsqueeze(1).broadcast_to((128, w_, 16, 16)), ALU.mult)
            v.tensor_reduce(val3[:, a:b, :], eqa, mybir.AxisListType.X, ALU.add)
            return finish_prep(a, b)

        # ---- histogram + transpose (alternate the two HWDGE queues) ----
        def hist_sb(cpool, sbk):
            for t in range(4):
                ti = 4 * sbk + t
                ct_ = cpool.tile([128, NBINS], FP16, tag="c")
                nc.gpsimd.local_scatter(
                    ct_[:], sval3[:, ti, :], sidx3[:, ti, :],
                    channels=128, num_elems=NBINS, num_idxs=18)
                eng = nc.sync if ti % 2 == 0 else nc.scalar
                eng.dma_start_transpose(
                    ct3[sbk][:, :, 16 + 128 * t: 144 + 128 * t], ct_[:])
            if sbk % SB_PER_BATCH == 0:
                v.memset(ct3[sbk][:, :, 15:16], 0.0)
            else:
                v.tensor_copy(ct3[sbk][:, :, 15:16], ct3[sbk - 1][:, :, 527:528])

        with tc.tile_pool(name="cp", bufs=4) as cpool:
            h1 = head_prep(0, 4)
            hist_sb(cpool, 0)
            h2 = head_prep(4, 8)
            hist_sb(cpool, 1)
            prep(8, NT, after=h2)
            for sbk in range(2, NSB):
                hist_sb(cpool, sbk)

            # ---- conv1 / conv2 / fc : software-pipelined over s-blocks ----
            with tc.tile_pool(name="p1", bufs=3, space="PSUM") as pp, \
                 tc.tile_pool(name="p2", bufs=2, space="PSUM") as pp2, \
                 tc.tile_pool(name="pf", bufs=3, space="PSUM") as pf, \
                 tc.tile_pool(name="ob", bufs=3) as ob:

                def conv1_sb(sbk):
                    if sbk % SB_PER_BATCH == SB_PER_BATCH - 1:
                        v.memset(ct3[sbk][:, :, 528:529], 0.0)
                    else:
                        v.tensor_copy(ct3[sbk][:, :, 528:529], ct3[sbk + 1][:, :, 16:17])
                    for co in range(2):
                        ps = pp.tile([128, 512], F32, tag="ps")
                        mms = [(1, 0)] + [(k, q) for k in range(3) for q in range(NQ)
                                          if not (k == 1 and q == 0)]
                        for i, (k, q) in enumerate(mms):
                            nc.tensor.matmul(
                                ps[:], f1t5[:, k, q, co, :],
                                ct3[sbk][:, q, 15 + k: 527 + k],
                                start=(i == 0), stop=(i == len(mms) - 1))
                        nc.scalar.activation(
                            x23[sbk][:, co, 16:528], ps[:],
                            mybir.ActivationFunctionType.Relu,
                            bias=s_b1[:, co:co + 1])
                    if sbk % SB_PER_BATCH == 0:
                        v.memset(x23[sbk][:, :, 15:16], 0.0)
                    else:
                        v.tensor_copy(x23[sbk][:, :, 15:16], x23[sbk - 1][:, :, 527:528])

                def conv2_sb(sbk):
                    if sbk % SB_PER_BATCH == SB_PER_BATCH - 1:
                        v.memset(x23[sbk][:, :, 528:529], 0.0)
                    else:
                        v.tensor_copy(x23[sbk][:, :, 528:529], x23[sbk + 1][:, :, 16:17])
                    ps2 = pp2.tile([64, 512], F32, tag="ps2")
                    mms = [(1, 0), (0, 0), (2, 0), (0, 1), (1, 1), (2, 1)]
                    for i, (k, q) in enumerate(mms):
                        nc.tensor.matmul(
                            ps2[:], w2t4[:, k, q, :],
                            x23[sbk][:, q, 15 + k: 527 + k],
                            start=(i == 0), stop=(i == len(mms) - 1))
                    nc.scalar.activation(
                        s_x3[0:64, 512 * sbk: 512 * (sbk + 1)], ps2[:],
                        mybir.ActivationFunctionType.Relu, bias=s_b2[:, 0:1])

                def fc_sb(sbk):
                    so = ob.tile([128, 4 * 152], F32, tag="o")
                    so3 = so[:].rearrange("p (t v) -> p t v", v=152)
                    for i, t in enumerate(range(4 * sbk, 4 * sbk + 4)):
                        psf = pf.tile([128, CHORD_SIZE], F32, tag="psf")
                        nc.tensor.matmul(psf[:], s_x3[:, 128 * t: 128 * (t + 1)],
                                         s_fcwb[:, 0:CHORD_SIZE], start=True, stop=True)
                        nc.scalar.copy(so3[:, i, 0:CHORD_SIZE], psf[:])
                    dst = d_out.ap()[512 * sbk: 512 * (sbk + 1), :].rearrange(
                        "(t p) v -> p t v", t=4)
                    nc.sync.dma_start(dst, so3[:, :, 0:CHORD_SIZE])

                for sbk in range(NSB):
                    conv1_sb(sbk)
                    if sbk >= 2:
                        conv2_sb(sbk - 2)
                    if sbk >= 3:
                        fc_sb(sbk - 3)
                for sbk in range(NSB - 2, NSB):
                    conv2_sb(sbk)
                for sbk in range(NSB - 3, NSB):
                    fc_sb(sbk)

    nc.compile()
    return nc


_NC = None


def _get_nc():
    global _NC
    if _NC is None:
        _NC = _build_program()
    return _NC


def _host_prep(chord_emb, note_emb, conv1_w, conv1_b, conv2_w, conv2_b, fc_w, fc_b):
    """Shared (replicated) constant tensors."""
    note_emb = np.asarray(note_emb, np.float32)
    chord_emb = np.asarray(chord_emb, np.float32)
    w1 = np.asarray(conv1_w, np.float32)
    E = np.zeros((NBINS, 512), np.float32)
    E[0:NOTE_SIZE, 256:512] = note_emb
    E[NOTE_SIZE:NOTE_SIZE + CHORD_SIZE, 0:256] = chord_emb
    F = np.einsum('vc,ock->kvo', E, w1)                      # [3, 1024, 256]
    f1t = F.reshape(3, NQ, 128, 2, 128).transpose(2, 0, 1, 3, 4)  # [128,3,q,c,o]
    f1t = np.ascontiguousarray(f1t, np.float16).reshape(128, -1)

    w2 = np.asarray(conv2_w, np.float32).reshape(64, 2, 128, 3)
    w2t = np.ascontiguousarray(w2.transpose(2, 3, 1, 0), np.float16).reshape(128, -1)

    fcwb = np.zeros((65, 152), np.float16)
    fcwb[0:64, 0:CHORD_SIZE] = np.asarray(fc_w, np.float16)
    fcwb[64, 0:CHORD_SIZE] = np.asarray(fc_b, np.float16)

    b1t = np.ascontiguousarray(
        np.asarray(conv1_b, np.float32).reshape(2, 128).T)
    b2t = np.asarray(conv2_b, np.float32).reshape(64, 1)

    jj = np.arange(16, dtype=np.int16)
    le = (jj[None, :] <= jj[:, None]).astype(np.int16).reshape(-1)   # j' <= j
    prepc = np.zeros((128, 528), np.int16)
    prepc[:, 0:256] = le[None, :]
    prepc[:, 512:528] = jj[None, :]
    return f1t, w2t, fcwb, b1t, b2t, prepc


def kernel(chord_emb, note_emb, conv1_w, conv1_b, conv2_w, conv2_b, fc_w, fc_b,
           note, chord):
    nc = _get_nc()
    f1t, w2t, fcwb, b1t, b2t, prepc = _host_prep(
        chord_emb, note_emb, conv1_w, conv1_b, conv2_w, conv2_b, fc_w, fc_b)

    note = np.asarray(note)
    chord = np.asarray(chord)
    in_maps = []
    for c in range(NCORES):
        nf = note[BLOC * c: BLOC * (c + 1)].reshape(P, N).astype(np.int16)
        cf = chord[BLOC * c: BLOC * (c + 1)].reshape(P).astype(np.int16) + NOTE_SIZE
        note16 = np.ascontiguousarray(
            nf.reshape(NT, 128, 16).transpose(1, 0, 2)).reshape(128, -1)
        chord16 = np.ascontiguousarray(cf.reshape(NT, 128).T)
        in_maps.append({
            "note16": note16, "chord16": chord16, "f1t": f1t, "w2t": w2t,
            "fcwb": fcwb, "b1t": b1t, "b2t": b2t, "prepc": prepc,
        })

    res = run_bass_kernel_spmd(nc, in_maps, list(range(NCORES)))
    outs = [res.results[c]["out"].reshape(BLOC, S, CHORD_SIZE)
            for c in range(NCORES)]
    return np.concatenate(outs, axis=0).astype(np.float32)



# revision 9
# speedup vs baseline: 1.3565x; 1.2638x over previous
"""Trainium2 Bass kernel for nn_Chord_Note_Conv (ragged embedding-bag + conv1d stack).

Design (per core, data-parallel over batch: B=16 -> 2 batch rows/core, P=4096 positions):
  * The ragged note-embedding mean, the chord embedding lookup, AND conv1 are fused
    into sparse-histogram matmuls: for each position build a 1024-bin count row
    C'[pos, v] (note bins 0..831 scaled by 1/cnt, chord bin 832+chord = 1),
    then conv1(x) = sum_k C'[s+k-1, :] @ F_k with F_k = E'' @ W1_k^T precomputed
    on host ([1024, 256] per tap).
  * C' rows are built with the GPSIMD local_scatter instruction. Duplicate notes
    are handled by scattering cumulative counts: the scatter applies indices in
    order, so the last occurrence of a value wins and leaves the total count.
  * conv1 runs in fp8 (e4m3) with DoubleRow perf mode (2 contraction chunks per
    instruction, 2x MAC rate). F tables are pre-scaled by 4096 on host; the
    conv1 activation un-scales by 1/4096. The fp16 histogram is cast to fp8 on
    DVE per s-block. Numpy-simulated rel err of the fp8 path: ~1.1e-3.
  * conv2 + fc are fp16 PE matmuls; fc bias via an appended ones-row.
"""

import os

os.environ.setdefault("MYCRO_LOCAL_CACHE", "1")

import numpy as np

try:
    import concourse.bass as bass  # noqa: F401
except ImportError:
    import sys

    sys.path.insert(0, "/opt/trn_rl_repo")
    import concourse.bass as bass

from concourse import bacc, mybir, tile
from concourse.bass_utils import run_bass_kernel_spmd

FP16 = mybir.dt.float16   # NB: bf16 produced wrong results on HW (and was no faster)
FP8 = mybir.dt.float8e4
F32 = mybir.dt.float32
I16 = mybir.dt.int16
ALU = mybir.AluOpType

CHORD_SIZE, NOTE_SIZE = 150, 832
B, S, N = 16, 2048, 16
NCORES = 8
BLOC = B // NCORES          # batch rows per core
P = BLOC * S                # positions per core = 4096
NT = P // 128               # pos tiles per core = 32
NSB = P // 512              # s-blocks per core = 8
SB_PER_BATCH = S // 512     # 4
NBINS = 1024                # 832 note + 150 chord + pad
NQ = NBINS // 128           # 8 v-chunks
CW = 544                    # ct/x2 s-block buffer width (16 guard + 512 + pad)
FSCALE = 4096.0             # fp8 pre-scale of the F tables (e4m3 max 240)


def _build_program():
    nc = bacc.Bacc("TRN2", target_bir_lowering=False, debug=False,
                   enable_asserts=False, num_devices=NCORES)

    # ---- DRAM I/O ----
    # note[0:512] + chord[512:544] + prep tables[544:1072] in ONE input tensor
    # (single DMA: each small DMA has ~3us completion latency on the queue).
    d_pin = nc.dram_tensor("pin", [128, 1072], I16, kind="ExternalInput")
    d_f1t = nc.dram_tensor("f1t", [128, 3 * NQ * 2 * 128], FP8, kind="ExternalInput")
    d_w2t = nc.dram_tensor("w2t", [128, 3 * 2 * 64], FP16, kind="ExternalInput")
    d_fcwb = nc.dram_tensor("fcwb", [65, 152], FP16, kind="ExternalInput")
    d_b1 = nc.dram_tensor("b1t", [128, 2], F32, kind="ExternalInput")
    d_b2 = nc.dram_tensor("b2t", [64, 1], F32, kind="ExternalInput")
    d_out = nc.dram_tensor("out", [P, CHORD_SIZE], F32, kind="ExternalOutput")

    # ---- persistent SBUF ----
    def sb(name, shape, dt):
        return nc.alloc_sbuf_tensor(name, list(shape), dt).ap()

    s_pin = sb("s_pin", [128, 1072], I16)
    s_f1t = sb("s_f1t", [128, 3 * NQ * 2 * 128], FP8)
    s_w2t = sb("s_w2t", [128, 3 * 2 * 64], FP16)
    s_fcwb = sb("s_fcwb", [65, 152], FP16)
    s_b1 = sb("s_b1", [128, 2], F32)
    s_b2 = sb("s_b2", [64, 1], F32)
    s_eqall = sb("s_eqall", [128, 4 * 256], I16)
    s_x3 = sb("s_x3", [65, P], FP16)
    s_mask = sb("s_mask", [128, NT * 16], I16)
    s_val = sb("s_val", [128, NT * 16], I16)
    s_eq = sb("s_eq", [128, NT * 16], I16)
    s_tmp = sb("s_tmp", [128, NT * 16], I16)
    s_v16 = sb("s_v16", [128, NT * 16], FP16)
    s_cnt = sb("s_cnt", [128, NT], F32)
    s_inv = sb("s_inv", [128, NT], F32)
    s_inv16 = sb("s_inv16", [128, NT], FP16)
    s_sidx = sb("s_sidx", [128, NT * 18], I16)
    s_sval = sb("s_sval", [128, NT * 18], FP16)
    # 8 ct s-block buffers [128, NQ, CW] fp16 + fp8 copies + 8 x2 buffers
    s_ct = [sb(f"s_ct{i}", [128, NQ * CW], FP16) for i in range(NSB)]
    s_ct8 = [sb(f"s_ct8{i}", [128, NQ * CW], FP8) for i in range(NSB)]
    s_x2 = [sb(f"s_x2{i}", [128, 2 * CW], FP16) for i in range(NSB)]

    s_note = s_pin[:, 0:512]
    s_chord = s_pin[:, 512:544]
    note3 = s_note.rearrange("p (c j) -> p c j", j=16)
    mask3 = s_mask.rearrange("p (c j) -> p c j", j=16)
    val3 = s_val.rearrange("p (c j) -> p c j", j=16)
    eq3 = s_eq.rearrange("p (c j) -> p c j", j=16)
    tmp3 = s_tmp.rearrange("p (c j) -> p c j", j=16)
    v163 = s_v16.rearrange("p (c j) -> p c j", j=16)
    sidx3 = s_sidx.rearrange("p (c j) -> p c j", j=18)
    sval3 = s_sval.rearrange("p (c j) -> p c j", j=18)
    f1t6 = s_f1t.rearrange("p (k qp co c o) -> p k qp co c o", k=3, qp=4, co=2, c=2)
    w2t4 = s_w2t.rearrange("p (k q o) -> p k q o", k=3, q=2)
    ct3 = [t.rearrange("p (q w) -> p q w", w=CW) for t in s_ct]
    ct83 = [t.rearrange("p (q w) -> p q w", w=CW) for t in s_ct8]
    x23 = [t.rearrange("p (q w) -> p q w", w=CW) for t in s_x2]

    with tile.TileContext(nc) as tc, \
         nc.allow_low_precision(reason="int16 counts <=16 are exact; fp8 conv1 "
                                       "sim rel err ~1.1e-3 vs 2e-2 budget"):
        v = nc.vector
        # ---- input loads: prep-critical single DMA on sync queue, weights on
        # the scalar (Activation) HWDGE queue in parallel ----
        nc.sync.dma_start(s_pin, d_pin.ap())
        nc.scalar.dma_start(s_f1t, d_f1t.ap())
        nc.scalar.dma_start(s_w2t, d_w2t.ap())
        nc.scalar.dma_start(s_fcwb, d_fcwb.ap())
        nc.scalar.dma_start(s_b1, d_b1.ap())
        nc.scalar.dma_start(s_b2, d_b2.ap())

        # ---- constants ----
        nc.gpsimd.memset(s_x3[64:65, :], 1.0)  # fc ones row (Pool is idle early)
        v.memset(sidx3[:, :, 17:18], -1)       # pad slot ignored
        v.memset(sval3[:, :, 16:17], 1.0)      # chord weight
        v.memset(sval3[:, :, 17:18], 0.0)

        # Duplicate notes: slot j scatters the cumulative count of note_j over
        # slots j' <= j; the scatter applies indices in order, so the last
        # occurrence wins and leaves the total count in the bin.
        def prep(a, b, after=None):
            w_ = b - a
            nt = note3[:, a:b, :]
            mk = mask3[:, a:b, :]
            vl = val3[:, a:b, :]
            eq = eq3[:, a:b, :]
            first = v.tensor_scalar(mk, nt, 0, None, ALU.not_equal)
            if after is not None:
                tile.add_dep_helper(first.ins, after.ins, sync=False,
                                    reason="bulk prep after head prep")
            for j in range(1, 16):
                v.tensor_tensor(mk[:, :, j], mk[:, :, j - 1], mk[:, :, j], ALU.mult)
            v.reduce_sum(s_cnt[:, a:b], mk, axis=mybir.AxisListType.X)
            v.tensor_scalar_max(s_cnt[:, a:b], s_cnt[:, a:b], 1.0)
            v.reciprocal(s_inv[:, a:b], s_cnt[:, a:b])
            v.tensor_copy(s_inv16[:, a:b], s_inv[:, a:b])
            v.tensor_copy(vl, mk)
            for d in range(1, 16):
                v.tensor_tensor(eq[:, :, d:], nt[:, :, d:], nt[:, :, :16 - d],
                                ALU.is_equal)
                v.tensor_tensor(vl[:, :, d:], vl[:, :, d:], eq[:, :, d:], ALU.add)
            return finish_prep(a, b)

        def finish_prep(a, b):
            """common tail: scat_idx, scat_val from mask/val."""
            w_ = b - a
            nt = note3[:, a:b, :]
            mk = mask3[:, a:b, :]
            eq = eq3[:, a:b, :]
            tp = tmp3[:, a:b, :]
            v.tensor_tensor(tp, nt, mk, ALU.mult)
            v.tensor_scalar_add(eq, mk, -1)
            v.tensor_tensor(sidx3[:, a:b, 0:16], tp, eq, ALU.add)
            v.tensor_copy(sidx3[:, a:b, 16:17], s_chord[:, a:b].unsqueeze(2))
            v.tensor_copy(v163[:, a:b, :], val3[:, a:b, :])
            inv_b = s_inv16[:, a:b].unsqueeze(2).broadcast_to((128, w_, 16))
            return v.tensor_tensor(sval3[:, a:b, 0:16], v163[:, a:b, :], inv_b, ALU.mult)

        le2 = s_pin[:, 544:800].rearrange("p (a b) -> p a b", b=16)  # 1[j' <= j]
        jidx = s_pin[:, 1056:1072]                                   # iota 0..15

        def head_prep(a, b):
            """Low-latency variant: 16x16 outer compare in few big ops."""
            w_ = b - a
            nt = note3[:, a:b, :]
            mk = mask3[:, a:b, :]
            eq = eq3[:, a:b, :]
            tp = tmp3[:, a:b, :]
            eqa = s_eqall.rearrange("p (c i j) -> p c i j", i=16, j=16)[:, 0:w_, :, :]
            jid_b = jidx.unsqueeze(1).broadcast_to((128, w_, 16))
            # first-zero index -> mask, cnt
            v.tensor_scalar(eq, nt, 0, None, ALU.not_equal)            # nz
            v.scalar_tensor_tensor(tp, eq, 16, jid_b, ALU.mult, ALU.add)
            v.tensor_reduce(s_cnt[:, a:b], tp, mybir.AxisListType.X, ALU.min)
            v.tensor_tensor(mk, jid_b,
                            s_cnt[:, a:b].unsqueeze(2).broadcast_to((128, w_, 16)),
                            ALU.is_lt)
            v.tensor_scalar(s_cnt[:, a:b], s_cnt[:, a:b], 16.0, 1.0, ALU.min, ALU.max)
            v.reciprocal(s_inv[:, a:b], s_cnt[:, a:b])
            v.tensor_copy(s_inv16[:, a:b], s_inv[:, a:b])
            # 16x16 equality outer product -> cumulative dup count over j' <= j
            v.tensor_tensor(eqa,
                            nt.unsqueeze(3).broadcast_to((128, w_, 16, 16)),
                            nt.unsqueeze(2).broadcast_to((128, w_, 16, 16)),
                            ALU.is_equal)
            v.tensor_tensor(eqa, eqa,
                            le2.unsqueeze(1).broadcast_to((128, w_, 16, 16)), ALU.mult)
            v.tensor_reduce(val3[:, a:b, :], eqa, mybir.AxisListType.X, ALU.add)
            return finish_prep(a, b)

        # ---- histogram + transpose (alternate the two HWDGE queues) ----
        def hist_sb(cpool, sbk):
            for t in range(4):
                ti = 4 * sbk + t
                ct_ = cpool.tile([128, NBINS], FP16, tag="c")
                nc.gpsimd.local_scatter(
                    ct_[:], sval3[:, ti, :], sidx3[:, ti, :],
                    channels=128, num_elems=NBINS, num_idxs=18)
                eng = nc.sync
                eng.dma_start_transpose(
                    ct3[sbk][:, :, 16 + 128 * t: 144 + 128 * t], ct_[:])
            if sbk % SB_PER_BATCH == 0:
                v.memset(ct3[sbk][:, :, 15:16], 0.0)
            else:
                v.tensor_copy(ct3[sbk][:, :, 15:16], ct3[sbk - 1][:, :, 527:528])

        with tc.tile_pool(name="cp", bufs=4) as cpool:
            h1 = head_prep(0, 4)
            hist_sb(cpool, 0)
            h2 = head_prep(4, 8)
            hist_sb(cpool, 1)
            prep(8, NT, after=h2)
            for sbk in range(2, NSB):
                hist_sb(cpool, sbk)

            # ---- conv1 / conv2 / fc : software-pipelined over s-blocks ----
            with tc.tile_pool(name="p1", bufs=3, space="PSUM") as pp, \
                 tc.tile_pool(name="p2", bufs=2, space="PSUM") as pp2, \
                 tc.tile_pool(name="pf", bufs=3, space="PSUM") as pf, \
                 tc.tile_pool(name="ob", bufs=3) as ob:

                def conv1_sb(sbk):
                    if sbk % SB_PER_BATCH == SB_PER_BATCH - 1:
                        v.memset(ct3[sbk][:, :, 528:529], 0.0)
                    else:
                        v.tensor_copy(ct3[sbk][:, :, 528:529], ct3[sbk + 1][:, :, 16:17])
                    # cast the full guarded window to fp8 for DoubleRow matmuls
                    v.tensor_copy(ct83[sbk][:, :, 15:529], ct3[sbk][:, :, 15:529])
                    for co in range(2):
                        ps = pp.tile([128, 512], F32, tag="ps")
                        mms = [(1, 0)] + [(k, qp) for k in range(3) for qp in range(4)
                                          if not (k == 1 and qp == 0)]
                        for i, (k, qp) in enumerate(mms):
                            nc.tensor.matmul(
                                ps[:], f1t6[:, k, qp, co, :, :],
                                ct83[sbk][:, 2 * qp: 2 * qp + 2, 15 + k: 527 + k],
                                start=(i == 0), stop=(i == len(mms) - 1),
                                perf_mode=mybir.MatmulPerfMode.DoubleRow)
                        nc.scalar.activation(
                            x23[sbk][:, co, 16:528], ps[:],
                            mybir.ActivationFunctionType.Relu,
                            bias=s_b1[:, co:co + 1], scale=1.0 / FSCALE)
                    if sbk % SB_PER_BATCH == 0:
                        v.memset(x23[sbk][:, :, 15:16], 0.0)
                    else:
                        v.tensor_copy(x23[sbk][:, :, 15:16], x23[sbk - 1][:, :, 527:528])

                def conv2_sb(sbk):
                    if sbk % SB_PER_BATCH == SB_PER_BATCH - 1:
                        v.memset(x23[sbk][:, :, 528:529], 0.0)
                    else:
                        v.tensor_copy(x23[sbk][:, :, 528:529], x23[sbk + 1][:, :, 16:17])
                    ps2 = pp2.tile([64, 512], F32, tag="ps2")
                    mms = [(1, 0), (0, 0), (2, 0), (0, 1), (1, 1), (2, 1)]
                    for i, (k, q) in enumerate(mms):
                        nc.tensor.matmul(
                            ps2[:], w2t4[:, k, q, :],
                            x23[sbk][:, q, 15 + k: 527 + k],
                            start=(i == 0), stop=(i == len(mms) - 1))
                    nc.scalar.activation(
                        s_x3[0:64, 512 * sbk: 512 * (sbk + 1)], ps2[:],
                        mybir.ActivationFunctionType.Relu, bias=s_b2[:, 0:1])

                def fc_sb(sbk):
                    so = ob.tile([128, 4 * 152], F32, tag="o")
                    so3 = so[:].rearrange("p (t v) -> p t v", v=152)
                    for i, t in enumerate(range(4 * sbk, 4 * sbk + 4)):
                        psf = pf.tile([128, CHORD_SIZE], F32, tag="psf")
                        nc.tensor.matmul(psf[:], s_x3[:, 128 * t: 128 * (t + 1)],
                                         s_fcwb[:, 0:CHORD_SIZE], start=True, stop=True)
                        nc.scalar.copy(so3[:, i, 0:CHORD_SIZE], psf[:])
                    dst = d_out.ap()[512 * sbk: 512 * (sbk + 1), :].rearrange(
                        "(t p) v -> p t v", t=4)
                    nc.sync.dma_start(dst, so3[:, :, 0:CHORD_SIZE])

                for sbk in range(NSB):
                    conv1_sb(sbk)
                    if sbk >= 2:
                        conv2_sb(sbk - 2)
                    if sbk >= 3:
                        fc_sb(sbk - 3)
                for sbk in range(NSB - 2, NSB):
                    conv2_sb(sbk)
                for sbk in range(NSB - 3, NSB):
                    fc_sb(sbk)

    nc.compile()
    return nc


_NC = None


def _get_nc():
    global _NC
    if _NC is None:
        _NC = _build_program()
    return _NC


NP_FP8 = mybir.dt.np(FP8)


def _host_prep(chord_emb, note_emb, conv1_w, conv1_b, conv2_w, conv2_b, fc_w, fc_b):
    """Shared (replicated) constant tensors."""
    note_emb = np.asarray(note_emb, np.float32)
    chord_emb = np.asarray(chord_emb, np.float32)
    w1 = np.asarray(conv1_w, np.float32)
    E = np.zeros((NBINS, 512), np.float32)
    E[0:NOTE_SIZE, 256:512] = note_emb
    E[NOTE_SIZE:NOTE_SIZE + CHORD_SIZE, 0:256] = chord_emb
    F = np.einsum('vc,ock->kvo', E, w1)                      # [3, 1024, 256]
    # [k, qp, c, vp, co, o] -> [vp, k, qp, co, c, o], pre-scaled for fp8
    f1t = (F * FSCALE).reshape(3, 4, 2, 128, 2, 128).transpose(3, 0, 1, 4, 2, 5)
    f1t = np.ascontiguousarray(f1t).astype(NP_FP8).reshape(128, -1)

    w2 = np.asarray(conv2_w, np.float32).reshape(64, 2, 128, 3)
    w2t = np.ascontiguousarray(w2.transpose(2, 3, 1, 0), np.float16).reshape(128, -1)

    fcwb = np.zeros((65, 152), np.float16)
    fcwb[0:64, 0:CHORD_SIZE] = np.asarray(fc_w, np.float16)
    fcwb[64, 0:CHORD_SIZE] = np.asarray(fc_b, np.float16)

    b1t = np.ascontiguousarray(
        np.asarray(conv1_b, np.float32).reshape(2, 128).T)
    b2t = np.asarray(conv2_b, np.float32).reshape(64, 1)

    jj = np.arange(16, dtype=np.int16)
    le = (jj[None, :] <= jj[:, None]).astype(np.int16).reshape(-1)   # j' <= j
    prepc = np.zeros((528,), np.int16)
    prepc[0:256] = le
    prepc[512:528] = jj
    return f1t, w2t, fcwb, b1t, b2t, prepc


def make_in_maps(chord_emb, note_emb, conv1_w, conv1_b, conv2_w, conv2_b,
                 fc_w, fc_b, note, chord):
    f1t, w2t, fcwb, b1t, b2t, prepc = _host_prep(
        chord_emb, note_emb, conv1_w, conv1_b, conv2_w, conv2_b, fc_w, fc_b)
    note = np.asarray(note)
    chord = np.asarray(chord)
    in_maps = []
    for c in range(NCORES):
        nf = note[BLOC * c: BLOC * (c + 1)].reshape(P, N).astype(np.int16)
        cf = chord[BLOC * c: BLOC * (c + 1)].reshape(P).astype(np.int16) + NOTE_SIZE
        pin = np.empty((128, 1072), np.int16)
        pin[:, 0:512] = np.ascontiguousarray(
            nf.reshape(NT, 128, 16).transpose(1, 0, 2)).reshape(128, -1)
        pin[:, 512:544] = cf.reshape(NT, 128).T
        pin[:, 544:1072] = prepc[None, :]
        in_maps.append({
            "pin": pin, "f1t": f1t, "w2t": w2t,
            "fcwb": fcwb, "b1t": b1t, "b2t": b2t,
        })
    return in_maps


def kernel(chord_emb, note_emb, conv1_w, conv1_b, conv2_w, conv2_b, fc_w, fc_b,
           note, chord):
    nc = _get_nc()
    in_maps = make_in_maps(chord_emb, note_emb, conv1_w, conv1_b, conv2_w,
                           conv2_b, fc_w, fc_b, note, chord)
    res = run_bass_kernel_spmd(nc, in_maps, list(range(NCORES)))
    outs = [res.results[c]["out"].reshape(BLOC, S, CHORD_SIZE)
            for c in range(NCORES)]
    return np.concatenate(outs, axis=0).astype(np.float32)
